# revision 1
# baseline (speedup 1.0000x reference)
"""Trainium2 Bass kernel for nn_AttentionModule (sparse_attention).

Computation (reference):
  q = tanh(einsum('hde,be->hbd', Query, x))          H=8 D=256 E=1536
  k = tanh(einsum('hdf,blf->hbld', Key, bank))       B=64 L=256 F=768
  s = einsum('hbld,hbd->hbl', k, q)  masked softmax over l
  out = LeakyReLU_0.4(einsum('hbl,blf->bhf', attn, bank))

Strategy: data-parallel over batch B across 8 NeuronCores (8 b's per core).
Host prep only re-lays-out inputs (transposes / mask bias); all FLOPs on
device.  The dominant k-matmul runs fp32r (full-rate, TF32-style rounding);
the small q / score paths run bf16 (tanh outputs are in [-1,1]).  Scores are
software-pipelined one b-pair behind the k-matmuls so the PE never waits on
the QueryT stream.
"""

import numpy as np
import ml_dtypes

import concourse.bass as bass  # noqa: F401
import concourse.mybir as mybir
import concourse.tile as tile
from concourse import bacc, bass_utils

F32 = mybir.dt.float32
F32R = mybir.dt.float32r
BF16 = mybir.dt.bfloat16
AF = mybir.ActivationFunctionType
AX = mybir.AxisListType

import os as _osd
ALLBF16 = _osd.environ.get("KERNEL_ALLBF16", "0") == "1"
MMDT = BF16 if ALLBF16 else F32R

H, D, E, F = 8, 256, 1536, 768
B, L = 64, 256
NCORES = 8
BPC = B // NCORES          # 8 b's per core
NBP = BPC // 2             # 4 b-pairs per core
EC, FC, DC, LC = E // 128, F // 128, D // 128, L // 128   # 12, 6, 2, 2


def _build_program():
    nc = bacc.Bacc("TRN2", target_bir_lowering=False, debug=False,
                   enable_asserts=False, num_devices=NCORES)
    qt = nc.dram_tensor("qt", [H, E, D], MMDT, kind="ExternalInput").ap()
    kt = nc.dram_tensor("kt", [H, F, D], MMDT, kind="ExternalInput").ap()
    bkt = nc.dram_tensor("bkt", [BPC, F, L], MMDT, kind="ExternalInput").ap()
    bkn = nc.dram_tensor("bkn", [BPC, L, F], MMDT, kind="ExternalInput").ap()
    xt = nc.dram_tensor("xt", [E, BPC], MMDT, kind="ExternalInput").ap()
    mb = nc.dram_tensor("mb", [BPC, H, L], F32, kind="ExternalInput").ap()
    eye = nc.dram_tensor("eye", [16, 16], F32, kind="ExternalInput").ap()
    zq = nc.dram_tensor("zq", [128, NBP * 640], MMDT, kind="ExternalInput").ap()
    out = nc.dram_tensor("out", [BPC, H, F], F32, kind="ExternalOutput").ap()

    with tile.TileContext(nc) as tc:
        with tc.tile_pool(name="const", bufs=1) as cpool, \
             tc.tile_pool(name="weights", bufs=1) as wpool, \
             tc.tile_pool(name="stream", bufs=2) as spool, \
             tc.tile_pool(name="kbuf", bufs=13) as kpool, \
             tc.tile_pool(name="small", bufs=2) as smpool, \
             tc.tile_pool(name="psA", bufs=3, space="PSUM") as psA, \
             tc.tile_pool(name="psB", bufs=2, space="PSUM") as psB, \
             tc.tile_pool(name="psS", bufs=3, space="PSUM") as psS:

            # ---- stream-tile loader (bkt split per fc for fast arrival) --
            def load_bkt(bp):
                bkt_t = spool.tile([128, FC * 512], MMDT, name="bkt_t", tag="bkt_t")
                v = bkt_t[:].rearrange("p (fc b l) -> p fc b l", fc=FC, b=2)
                for fc in range(FC):
                    nc.sync.dma_start(
                        v[:, fc],
                        bkt[2 * bp:2 * bp + 2, fc * 128:(fc + 1) * 128, :]
                        .rearrange("b p l -> p b l"))
                return bkt_t

            def load_bkn_mb(bp):
                bkn_ts = []
                for b2 in range(2):
                    bkn_t = spool.tile([128, LC * F], MMDT,
                                       name=f"bkn_t{b2}", tag=f"bkn_t{b2}")
                    nc.sync.dma_start(
                        bkn_t[:].rearrange("p (lc f) -> p lc f", lc=LC),
                        bkn[2 * bp + b2].rearrange("(lc p) f -> p lc f", p=128))
                    bkn_ts.append(bkn_t)
                mb_ts = []
                for b2 in range(2):
                    mb_t = smpool.tile([8, L], F32, name=f"mb_t{b2}", tag=f"mb_t{b2}")
                    nc.sync.dma_start(mb_t[:], mb[2 * bp + b2])
                    mb_ts.append(mb_t)
                return bkn_ts, mb_ts

            def load_bp_tiles(bp):
                bkt_t = load_bkt(bp)
                bkn_ts, mb_ts = load_bkn_mb(bp)
                return bkt_t, bkn_ts, mb_ts

            # KeyT, all heads, stays resident:  [128, fc*256 + d].
            # kt[0] + bp0's bank tiles are issued first so the PE can start
            # within a couple of microseconds; everything else streams behind.
            kt_tiles = []
            for h in range(H):
                t = wpool.tile([128, FC * D], MMDT, name=f"kt_sb{h}", tag=f"kt_sb{h}")
                kt_tiles.append(t)

            def load_kt(h):
                for piece in range(2):
                    nc.sync.dma_start(
                        kt_tiles[h][:, piece * (FC // 2) * D:
                                    (piece + 1) * (FC // 2) * D]
                        .rearrange("p (fc d) -> p fc d", fc=FC // 2),
                        kt[h, piece * (F // 2):(piece + 1) * (F // 2)]
                        .rearrange("(fc p) d -> p fc d", p=128))

            bkt0_t = spool.tile([128, FC * 512], MMDT, name="bkt_t", tag="bkt_t")
            v0 = bkt0_t[:].rearrange("p (fc b l) -> p fc b l", fc=FC, b=2)

            def load_bkt0_fc(fc):
                nc.sync.dma_start(
                    v0[:, fc],
                    bkt[0:2, fc * 128:(fc + 1) * 128, :].rearrange("b p l -> p b l"))

            import os as _os0
            REPEAT = int(_os0.environ.get("KERNEL_REPEAT", "1"))
            load_bkt0_fc(0)
            for piece in range(3):
                nc.sync.dma_start(
                    kt_tiles[0][:, piece * 2 * D:(piece + 1) * 2 * D]
                    .rearrange("p (fc d) -> p fc d", fc=2),
                    kt[0, piece * 256:(piece + 1) * 256]
                    .rearrange("(fc p) d -> p fc d", p=128))
            for fc in range(1, FC):
                load_bkt0_fc(fc)
            bkt0 = bkt0_t
            for h in range(1, H):
                load_kt(h)
            preloaded = {0: (bkt0, None, None)}

            eye_t = cpool.tile([16, 16], F32)
            xt_sb = cpool.tile([128, EC * BPC], MMDT)
            qz_sb = cpool.tile([128, NBP * 640], MMDT)

            def load_consts():
                nc.sync.dma_start(eye_t[:], eye)
                nc.sync.dma_start(
                    xt_sb[:].rearrange("p (ec b) -> p ec b", ec=EC),
                    xt.rearrange("(ec p) b -> p ec b", p=128))
                nc.sync.dma_start(qz_sb[:], zq)

            def q_phase(heads):
                """q = tanh(x @ Query^T): per h, psum[b=8, d=256] over 12
                E-chunks, then PE-transpose into the zero-padded score lhsT."""
                for h in heads:
                    pq = psS.tile([BPC, D], F32, name="pq", tag="pss")
                    for half in range(2):
                        qt_c = spool.tile([128, EC * D // 2], MMDT,
                                          name="qt_c", tag="qt_c")
                        nc.sync.dma_start(
                            qt_c[:].rearrange("p (ec d) -> p ec d", ec=EC // 2),
                            qt[h, half * (E // 2):(half + 1) * (E // 2)]
                            .rearrange("(ec p) d -> p ec d", p=128))
                        for e2 in range(EC // 2):
                            ec = half * (EC // 2) + e2
                            nc.tensor.matmul(pq[:], xt_sb[:, ec * BPC:(ec + 1) * BPC],
                                             qt_c[:, e2 * D:(e2 + 1) * D],
                                             start=(ec == 0), stop=(ec == EC - 1))
                    q_sb = smpool.tile([BPC, D], F32, name="q_sb", tag="q_sb")
                    nc.scalar.activation(q_sb[:], pq[:], AF.Tanh)
                    for dc in range(DC):
                        pt = psS.tile([128, BPC], F32, name="pt", tag="pss")
                        nc.tensor.transpose(pt[:], q_sb[:, dc * 128:(dc + 1) * 128],
                                            eye_t[0:BPC, 0:BPC])
                        for bp in range(NBP):
                            for b2 in range(2):
                                col = bp * 640 + (2 * h + dc) * 40 + 32 * b2 + h
                                nc.vector.tensor_copy(
                                    qz_sb[:, col:col + 1],
                                    pt[:, bp * 2 + b2:bp * 2 + b2 + 1])

            def compute_k(bp, bkt_t):
                """k = tanh(KeyT^T @ bankT) for all heads of this b-pair."""
                k_tiles = []
                for h in range(H):
                    k_t = kpool.tile([128, DC * 512], MMDT, name="k_t", tag="k_t")
                    for dc in range(DC):
                        pk = psA.tile([128, 512], F32, name="pk", tag="pk")
                        for fc in range(FC):
                            nc.tensor.matmul(
                                pk[:],
                                kt_tiles[h][:, fc * D + dc * 128:
                                            fc * D + dc * 128 + 128],
                                bkt_t[:, fc * 512:(fc + 1) * 512],
                                start=(fc == 0), stop=(fc == FC - 1))
                        nc.scalar.activation(k_t[:, dc * 512:(dc + 1) * 512], pk[:],
                                             AF.Tanh)
                    k_tiles.append(k_t)
                return k_tiles

            def score_phase(bp, k_tiles, bkn_ts, mb_ts, ps40=None):
                # score: accumulate all (h, dc) into one [40, 512] psum
                # (rows b2*32+h; cols 8..31 of each lhsT block are zero)
                if ps40 is None:
                    ps40 = psB.tile([40, 512], F32, name="ps40", tag="ps40")
                    for h in range(H):
                        for dc in range(DC):
                            base = bp * 640 + (2 * h + dc) * 40
                            nc.tensor.matmul(
                                ps40[:],
                                qz_sb[:, base:base + 40],
                                k_tiles[h][:, dc * 512:(dc + 1) * 512],
                                start=(h == 0 and dc == 0),
                                stop=(h == H - 1 and dc == DC - 1))

                # masked softmax over l (free axis); per-b2 tiles at base 0
                pT = smpool.tile([128, 32], MMDT, name="pT", tag="pT")
                rzs = []
                for b2 in range(2):
                    s_sb = smpool.tile([8, L], F32, name=f"s_sb{b2}", tag=f"s_sb{b2}")
                    nc.vector.tensor_add(s_sb[:],
                                         ps40[32 * b2:32 * b2 + 8,
                                              256 * b2:256 * b2 + 256],
                                         mb_ts[b2][:])
                    nmax = smpool.tile([8, 1], F32, name=f"nmax{b2}", tag=f"nmax{b2}")
                    nc.vector.reduce_max(nmax[:], s_sb[:], axis=AX.X, negate=True)
                    p_sb = smpool.tile([8, L], F32, name=f"p_sb{b2}", tag=f"p_sb{b2}")
                    zsum = smpool.tile([8, 1], F32, name=f"zsum{b2}", tag=f"zsum{b2}")
                    nc.scalar.activation(p_sb[:], s_sb[:], AF.Exp, bias=nmax[:],
                                         accum_out=zsum[:])
                    rz = smpool.tile([8, 1], F32, name=f"rz{b2}", tag=f"rz{b2}")
                    nc.vector.reciprocal(rz[:], zsum[:])
                    rzs.append(rz)
                    for lc in range(LC):
                        ptp = psS.tile([128, 8], F32, name="ptp", tag="pss")
                        nc.tensor.transpose(ptp[:], p_sb[:, lc * 128:(lc + 1) * 128],
                                            eye_t[0:8, 0:8])
                        nc.vector.tensor_copy(
                            pT[:, b2 * 16 + lc * 8:b2 * 16 + lc * 8 + 8], ptp[:])

                # emb = attn @ bank, normalize+LeakyReLU fused into Prelu
                import os as _os2
                simsafe = _os2.environ.get("KERNEL_SIM_SAFE", "0") == "1"
                for b2 in range(2):
                    o_sb = smpool.tile([8, F], F32, name=f"o_sb{b2}", tag=f"o_sb{b2}")
                    for fh in range(2):
                        pe = psS.tile([8, 384], F32, name="pe", tag="pss")
                        for lc in range(LC):
                            nc.tensor.matmul(
                                pe[:],
                                pT[:, b2 * 16 + lc * 8:b2 * 16 + lc * 8 + 8],
                                bkn_ts[b2][:, lc * F + fh * 384:
                                            lc * F + fh * 384 + 384],
                                start=(lc == 0), stop=(lc == LC - 1))
                        if simsafe:
                            nc.scalar.activation(o_sb[:, fh * 384:fh * 384 + 384],
                                                 pe[:], AF.Copy, scale=rzs[b2][:])
                        else:
                            nc.scalar.activation(o_sb[:, fh * 384:fh * 384 + 384],
                                                 pe[:], AF.Prelu,
                                                 scale=rzs[b2][:], alpha=0.4)
                    nc.sync.dma_start(out[2 * bp + b2], o_sb[:])

            # ---- main loop: scores pipelined one b-pair behind k ---------
            import os as _os
            PIPELINE = _os.environ.get("KERNEL_NO_PIPE", "0") != "1"
            for rep in range(REPEAT):
              if rep > 0:
                # re-stream everything, same work per repeat
                for h in range(H):
                    load_kt(h)
                preloaded = {0: load_bp_tiles(0)}
              pending = None
              for bp in range(NBP):
                  bkt_t, bkn_ts, mb_ts = preloaded.pop(bp)
                  if bkn_ts is None:
                      bkn_ts, mb_ts = load_bkn_mb(bp)
                  if bp + 1 < NBP:
                      preloaded[bp + 1] = load_bp_tiles(bp + 1)
                  if bp == 0:
                      load_consts()
                  k_tiles = compute_k(bp, bkt_t)
                  if bp == 0:
                      q_phase(range(0, 4))
                  elif bp == 1:
                      q_phase(range(4, 8))
                  if not PIPELINE:
                      score_phase(bp, k_tiles, bkn_ts, mb_ts)
                      continue
                  if pending is not None:
                      score_phase(*pending)
                  pending = (bp, k_tiles, bkn_ts, mb_ts)
              if PIPELINE:
                  score_phase(*pending)

    nc.finalize()
    return nc


def _host_prep(x, bank, mask, Query, Key):
    x = np.ascontiguousarray(x, dtype=np.float32)
    bank = np.ascontiguousarray(bank, dtype=np.float32)
    Query = np.ascontiguousarray(Query, dtype=np.float32)
    Key = np.ascontiguousarray(Key, dtype=np.float32)

    mmdt = ml_dtypes.bfloat16 if ALLBF16 else np.float32
    qt = np.ascontiguousarray(Query.transpose(0, 2, 1)).astype(mmdt)
    kt = np.ascontiguousarray(Key.transpose(0, 2, 1)).astype(mmdt)
    bkt = np.ascontiguousarray(bank.transpose(0, 2, 1)).astype(mmdt)
    bkn = bank.astype(mmdt)
    mbias = np.where(mask == 0, np.float32(-1e8), np.float32(0.0)).astype(np.float32)
    mb = np.ascontiguousarray(np.repeat(mbias[:, None, :], H, axis=1))
    eye = np.eye(16, dtype=np.float32)
    zq = np.zeros((128, NBP * 640), dtype=mmdt)

    in_maps = []
    for c in range(NCORES):
        bs = c * BPC
        in_maps.append({
            "qt": qt,
            "kt": kt,
            "bkt": np.ascontiguousarray(bkt[bs:bs + BPC]),
            "bkn": np.ascontiguousarray(bkn[bs:bs + BPC]),
            "xt": np.ascontiguousarray(x[bs:bs + BPC].T).astype(mmdt),
            "mb": np.ascontiguousarray(mb[bs:bs + BPC]),
            "eye": eye,
            "zq": zq,
        })
    return in_maps


_NC_CACHE = {}


def kernel(x, bank, mask, Query, Key):
    import os
    if "nc" not in _NC_CACHE:
        _NC_CACHE["nc"] = _build_program()
    nc = _NC_CACHE["nc"]
    in_maps = _host_prep(x, bank, mask, Query, Key)

    trace = os.environ.get("KERNEL_TRACE", "0") == "1"
    res = bass_utils.run_bass_kernel_spmd(nc, in_maps,
                                          core_ids=list(range(NCORES)),
                                          trace=trace)
    if trace:
        print("exec_time_ns:", res.exec_time_ns,
              "mean:", res.mean_exec_time_ns,
              "core:", res.max_exec_time_core_id)
    return np.concatenate([r["out"] for r in res.results], axis=0)



# revision 7
# speedup vs baseline: 2.2651x; 2.2651x over previous
"""Trainium2 Bass kernel for nn_AttentionModule (sparse_attention).

Reference computation:
  q = tanh(einsum('hde,be->hbd', Query, x))          H=8 D=256 E=1536
  k = tanh(einsum('hdf,blf->hbld', Key, bank))       B=64 L=256 F=768
  s = einsum('hbld,hbd->hbl', k, q)  masked softmax over l
  out = LeakyReLU_0.4(einsum('hbl,blf->bhf', attn, bank))

Strategy (data-parallel over batch B, 8 b's per core):
 * Mask compaction: the 0/1 mask keeps <=147 of 256 bank columns per b, so
   the host gathers unmasked columns and pads to LP=148.  Padding columns
   get a -1e4 additive score bias (exp -> 0) injected as an extra matmul.
 * The dominant k-matmul runs as error-compensated fp8 (e4m3): with
   Key*32 ~ K8 + Kr and bank*8 ~ B8 + Br, kraw = K8B8 + K8Br + KrB8
   (the fp8*fp8 residual cross term is negligible).  All three terms share
   one power-of-two scale, folded into the tanh eviction's `scale`.  Each
   product pair runs as a DoubleRow matmul (2 K-tiles per instruction).
 * Narrow dims (batch 8, heads 8) ride in the moving dimension: q, score,
   and emb matmuls cost ap_size 8 or 1 per instruction instead of 256-512.
 * Softmax skips max-subtraction (|score| < 40, safe in f32) so scores can
   stay in [l, h] layout; 1/z is broadcast to [f, h] via a ones-matmul and
   applied together with LeakyReLU on the vector engine.
 * All DMA streams are host-pre-swizzled to the exact SBUF layout
   ([128, X] row-major, contiguous >=512B lines); outputs are gathered as
   [f, (b2, fc, h)] tiles and transposed on the host.
"""

import os
import numpy as np
import ml_dtypes

import concourse.bass as bass  # noqa: F401
import concourse.mybir as mybir
import concourse.tile as tile
from concourse import bacc, bass_utils

F32 = mybir.dt.float32
F16 = mybir.dt.float16
BF16 = mybir.dt.bfloat16
FP8 = mybir.dt.float8e4
AF = mybir.ActivationFunctionType
ALU = mybir.AluOpType
DR = mybir.MatmulPerfMode.DoubleRow

H, D, E, F = 8, 256, 1536, 768
B, L = 64, 256
NCORES = 8
BPC = B // NCORES          # 8 b's per core
NBP = BPC // 2             # 4 b-pairs per core
EC, FC, DC = E // 128, F // 128, D // 128   # 12, 6, 2
LP_DEFAULT = 152           # padded unmasked-column count (runtime input max: 151)
SK, SB = 32.0, 8.0         # fp8 pre-scales for Key / bank (powers of two)

# f16 fallback for the k-matmul (accuracy reference / debugging)
K16 = os.environ.get("KERNEL_K16", "0") == "1"


def _build_program(lp=LP_DEFAULT):
    assert lp % 2 == 0
    lh = lp // 2                       # l-chunk (74): two chunks per b
    lpp = 2 * lp                       # (b2, l') columns per (h, dc) group
    nsk = 1 if K16 else 2              # fp8: [K8, Kr] / [Br, B8] stream pairs
    ktdt = F16 if K16 else FP8
    kt_cols = nsk * FC * D             # per-h Key cols
    bkt_cols = nsk * FC * lpp          # per-bp bankT cols
    tanh_scale = 1.0 if K16 else 1.0 / (SK * SB)

    nc = bacc.Bacc("TRN2", target_bir_lowering=False, debug=False,
                   enable_asserts=False, num_devices=NCORES)
    qt = nc.dram_tensor("qt", [H, 128, EC * D], F16, kind="ExternalInput").ap()
    xt = nc.dram_tensor("xt", [128, EC * BPC], F16, kind="ExternalInput").ap()
    kt = nc.dram_tensor("kt", [H, 128, kt_cols], ktdt, kind="ExternalInput").ap()
    bkt = nc.dram_tensor("bkt", [NBP, 128, bkt_cols], ktdt, kind="ExternalInput").ap()
    bkn = nc.dram_tensor("bkn", [NBP, 2, lh, 2 * F], BF16, kind="ExternalInput").ap()
    sbias = nc.dram_tensor("sbias", [1, NBP * 4 * lh], F32, kind="ExternalInput").ap()
    out = nc.dram_tensor("out", [NBP, 128, 2 * FC * H], F32, kind="ExternalOutput").ap()

    with tile.TileContext(nc) as tc:
        with tc.tile_pool(name="const", bufs=1) as cpool, \
             tc.tile_pool(name="weights", bufs=1) as wpool, \
             tc.tile_pool(name="bktp", bufs=2) as bpool, \
             tc.tile_pool(name="bknp", bufs=4) as npool, \
             tc.tile_pool(name="ksb", bufs=1) as kpool, \
             tc.tile_pool(name="small", bufs=2) as spool, \
             tc.tile_pool(name="psK", bufs=2, space="PSUM") as psK, \
             tc.tile_pool(name="psQ", bufs=1, space="PSUM") as psQ, \
             tc.tile_pool(name="psS", bufs=2, space="PSUM") as psS:

            # ---------------- DMA: priority order -------------------------
            xt_sb = cpool.tile([128, EC * BPC], F16)
            kt_sb = [wpool.tile([128, kt_cols], ktdt, name=f"kt{h}", tag=f"kt{h}")
                     for h in range(H)]
            qt_sb = [wpool.tile([128, EC * D], F16, name=f"qt{h}", tag=f"qt{h}")
                     for h in range(H)]
            bkt_t = [bpool.tile([128, bkt_cols], ktdt, name="bkt", tag="bkt")
                     for _ in range(NBP)]
            bkn_t = [[npool.tile([lh, 2 * F], BF16, name=f"bkn{b2}", tag=f"bkn{b2}")
                      for b2 in range(2)] for _ in range(NBP)]
            sb_sb = cpool.tile([1, NBP * 4 * lh], F32)
            onesb = cpool.tile([1, BPC], F32)
            ones_col = cpool.tile([lh, 1], BF16)
            ones128 = cpool.tile([1, 128], F32)

            nc.sync.dma_start(xt_sb[:], xt)
            nc.sync.dma_start(kt_sb[0][:], kt[0])
            nc.sync.dma_start(bkt_t[0][:], bkt[0])
            for h in range(1, H):
                nc.sync.dma_start(kt_sb[h][:], kt[h])
            nc.vector.memset(onesb[:], 1.0)
            nc.vector.memset(ones_col[:], 1.0)
            nc.vector.memset(ones128[:], 1.0)
            for bp in range(1, NBP):
                nc.sync.dma_start(bkt_t[bp][:], bkt[bp])
            for h in range(H):
                nc.sync.dma_start(qt_sb[h][:], qt[h])
            nc.sync.dma_start(sb_sb[:], sbias)
            for bp in range(NBP):
                for b2 in range(2):
                    nc.sync.dma_start(bkn_t[bp][b2][:], bkn[bp, b2])

            # ---------------- k = tanh(Key @ bankT), all bps --------------
            k_sb = {}

            def k_phase(bp):
                vb = bkt_t[bp][:].rearrange("p (s ft c) -> p s ft c", s=nsk, ft=FC)
                for h in range(H):
                    vk = kt_sb[h][:].rearrange("p (s ft d) -> p s ft d", s=nsk, ft=FC)
                    ps = psK.tile([128, 1024], F32, name="psk", tag="psk")
                    for dc in range(DC):
                        g = ps[:, dc * 512:dc * 512 + lpp]
                        if K16:
                            for ft in range(FC):
                                nc.tensor.matmul(
                                    g, vk[:, 0, ft, dc * 128:(dc + 1) * 128],
                                    vb[:, 0, ft], start=(ft == 0),
                                    stop=(ft == FC - 1))
                        else:
                            # T1: K8.B8 over f-tile pairs
                            for p in range(FC // 2):
                                nc.tensor.matmul(
                                    g,
                                    vk[:, 0, 2 * p:2 * p + 2, dc * 128:(dc + 1) * 128],
                                    vb[:, 1, 2 * p:2 * p + 2],
                                    start=(p == 0), stop=False, perf_mode=DR)
                            # cross terms: K8.Br + Kr.B8 per f-tile
                            for ft in range(FC):
                                nc.tensor.matmul(
                                    g,
                                    vk[:, :, ft, dc * 128:(dc + 1) * 128],
                                    vb[:, :, ft],
                                    start=False, stop=(ft == FC - 1), perf_mode=DR)
                    kt_out = kpool.tile([128, 2 * lpp], F16,
                                        name=f"k{bp}_{h}", tag=f"k{bp}_{h}")
                    nc.scalar.activation(
                        kt_out[:].rearrange("p (a b) -> p a b", a=2),
                        ps[:].rearrange("p (a b) -> p a b", a=2)[:, :, 0:lpp],
                        AF.Tanh, scale=tanh_scale)
                    k_sb[(bp, h)] = kt_out

            for bp in range(NBP):
                k_phase(bp)

            # ---------------- q = tanh(Query @ x), transposed -------------
            psq = psQ.tile([128, 128], F32)
            for h in range(H):
                vq = qt_sb[h][:].rearrange("p (ec d) -> p ec d", ec=EC)
                for dc in range(DC):
                    g = psq[:, (h * DC + dc) * BPC:(h * DC + dc + 1) * BPC]
                    for ec in range(EC):
                        nc.tensor.matmul(
                            g, vq[:, ec, dc * 128:(dc + 1) * 128],
                            xt_sb[:, ec * BPC:(ec + 1) * BPC],
                            start=(ec == 0), stop=(ec == EC - 1))
            q_sb = cpool.tile([128, 128], F16)
            nc.scalar.activation(q_sb[:], psq[:], AF.Tanh)

            # ---------------- score / softmax / emb per bp ----------------
            def score_emb(bp):
                ps = psS.tile([128, 512], F32, name="mix", tag="mix")
                # scores: out [l' 74, (b2, lc, h)], accumulate dc + pad bias
                for b2 in range(2):
                    for lc in range(2):
                        col = (b2 * 2 + lc) * H
                        boff = ((bp * 2 + b2) * 2 + lc) * lh
                        nc.tensor.matmul(ps[0:lh, col:col + H],
                                         sb_sb[:, boff:boff + lh],
                                         onesb[:], start=True, stop=False)
                        for h in range(H):
                            for dc in range(DC):
                                nc.tensor.matmul(
                                    ps[0:lh, col + h:col + h + 1],
                                    k_sb[(bp, h)][:, dc * lpp + b2 * lp +
                                                  lc * lh:dc * lpp + b2 * lp +
                                                  lc * lh + lh],
                                    q_sb[:, (h * DC + dc) * BPC + bp * 2 + b2:
                                         (h * DC + dc) * BPC + bp * 2 + b2 + 1],
                                    start=False,
                                    stop=(h == H - 1 and dc == DC - 1))
                exp_t = spool.tile([lh, 4 * H], BF16, name="exp", tag="exp")
                nc.scalar.activation(exp_t[:], ps[0:lh, 0:4 * H], AF.Exp)
                # z[b2, h] (cols 32:48): accumulate both lc chunks via
                # strided rhs slices so no cross-psum adds are needed
                ev = exp_t[:].rearrange("p (b2 lc h) -> p b2 lc h", b2=2, lc=2)
                for lc in range(2):
                    nc.tensor.matmul(ps[0:1, 32:48], ones_col[:], ev[:, :, lc],
                                     start=(lc == 0), stop=(lc == 1))
                rz = spool.tile([1, 2 * H], F32, name="rz", tag="rz")
                nc.vector.reciprocal(rz[:], ps[0:1, 32:48])
                rzr = spool.tile([1, 2 * FC * H], F32, name="rzr", tag="rzr")
                for b2 in range(2):
                    for fc in range(FC):
                        nc.vector.tensor_copy(
                            rzr[:, (b2 * FC + fc) * H:(b2 * FC + fc + 1) * H],
                            rz[:, b2 * H:(b2 + 1) * H])
                # rzb[f, (b2, fc, h)] broadcast (cols 96:192)
                nc.tensor.matmul(ps[:, 96:192], ones128[:], rzr[:],
                                 start=True, stop=True)
                # emb[f, (b2, fc, h)] (cols 192:288)
                for b2 in range(2):
                    for fc in range(FC):
                        col = 192 + (b2 * FC + fc) * H
                        for lc in range(2):
                            nc.tensor.matmul(
                                ps[:, col:col + H],
                                bkn_t[bp][b2][:, lc * F + fc * 128:
                                              lc * F + fc * 128 + 128],
                                exp_t[:, (b2 * 2 + lc) * H:(b2 * 2 + lc + 1) * H],
                                start=(lc == 0), stop=(lc == 1))
                # normalize + LeakyReLU on DVE, then store
                rzb_sb = spool.tile([128, 2 * FC * H], F32, name="rzb", tag="rzb")
                o1 = spool.tile([128, 2 * FC * H], F32, name="o1", tag="o1")
                o2 = spool.tile([128, 2 * FC * H], F32, name="o2", tag="o2")
                nc.vector.tensor_copy(rzb_sb[:], ps[:, 96:192])
                nc.vector.tensor_mul(o1[:], ps[:, 192:288], rzb_sb[:])
                nc.vector.scalar_tensor_tensor(o2[:], o1[:], 0.4, o1[:],
                                               ALU.mult, ALU.max)
                nc.sync.dma_start(out[bp], o2[:])

            for bp in range(NBP):
                score_emb(bp)

    nc.finalize()
    return nc


def _host_prep(x, bank, mask, Query, Key, lp):
    lh = lp // 2
    x = np.asarray(x, dtype=np.float32)
    bank = np.asarray(bank, dtype=np.float32)
    mask = np.asarray(mask)
    Query = np.asarray(Query, dtype=np.float32)
    Key = np.asarray(Key, dtype=np.float32)
    e4 = ml_dtypes.float8_e4m3

    # compact unmasked bank columns to LP, record padding bias
    bankc = np.zeros((B, lp, F), dtype=np.float32)
    bias = np.zeros((B, lp), dtype=np.float32)
    for b in range(B):
        idx = np.nonzero(mask[b])[0]
        bankc[b, :len(idx)] = bank[b, idx]
        bias[b, len(idx):] = -10000.0

    # q path: f16, host-transposed
    qt = np.ascontiguousarray(Query.transpose(0, 2, 1)).reshape(
        H, EC, 128, D).transpose(0, 2, 1, 3).reshape(H, 128, EC * D)
    qt = qt.astype(np.float16)

    def swz_key(Kt):  # [H, D, F] -> [H, 128(f), FC, D]
        t = np.ascontiguousarray(Kt.transpose(0, 2, 1))         # [H, F, D]
        return t.reshape(H, FC, 128, D).transpose(0, 2, 1, 3)

    def swz_bank(Bc):  # [B, LP, F] -> [B//2, 128(f), FC, 2, LP]
        t = np.ascontiguousarray(Bc.transpose(0, 2, 1))          # [B, F, LP]
        t = t.reshape(B // 2, 2, FC, 128, lp)
        return t.transpose(0, 3, 2, 1, 4)

    if K16:
        kt = swz_key(Key).reshape(H, 128, FC * D).astype(np.float16)
        bkt = swz_bank(bankc).reshape(B // 2, 128, FC * 2 * lp).astype(np.float16)
    else:
        Ks = Key * SK
        K8 = Ks.astype(e4)
        Kr = (Ks - K8.astype(np.float32)).astype(e4)
        kt = np.stack([swz_key(K8.astype(np.float32)),
                       swz_key(Kr.astype(np.float32))], axis=2)
        kt = kt.reshape(H, 128, 2 * FC * D).astype(e4)
        Bs = bankc * SB
        B8 = Bs.astype(e4)
        Br = (Bs - B8.astype(np.float32)).astype(e4)
        # s-order [Br, B8] so the cross-term AP pairs (K8,Br),(Kr,B8)
        bkt = np.stack([swz_bank(Br.astype(np.float32)),
                        swz_bank(B8.astype(np.float32))], axis=2)
        bkt = bkt.reshape(B // 2, 128, 2 * FC * 2 * lp).astype(e4)

    bkn = bankc.reshape(B, 2, lh, F)                    # [B, lc, lh, F]
    bkn = bkn.transpose(0, 2, 1, 3).reshape(B // 2, 2, lh, 2 * F)
    bkn = np.ascontiguousarray(bkn).astype(ml_dtypes.bfloat16)

    sbias = bias.reshape(B, 2, lh).reshape(NCORES, NBP * 4 * lh)[:, None, :]

    in_maps = []
    for c in range(NCORES):
        bs = c * BPC
        in_maps.append({
            "qt": qt,
            "xt": np.ascontiguousarray(
                x[bs:bs + BPC].T.reshape(EC, 128, BPC).transpose(1, 0, 2)
                .reshape(128, EC * BPC)).astype(np.float16),
            "kt": kt,
            "bkt": np.ascontiguousarray(bkt[bs // 2:bs // 2 + NBP]),
            "bkn": np.ascontiguousarray(bkn[bs // 2:bs // 2 + NBP]),
            "sbias": np.ascontiguousarray(sbias[c]),
        })
    return in_maps


_NC_CACHE = {}


def kernel(x, bank, mask, Query, Key):
    mask = np.asarray(mask)
    maxc = int(mask.sum(axis=1).max())
    lp = max(LP_DEFAULT, 2 * ((maxc + 1) // 2))
    if lp not in _NC_CACHE:
        _NC_CACHE[lp] = _build_program(lp)
    nc = _NC_CACHE[lp]
    in_maps = _host_prep(x, bank, mask, Query, Key, lp)

    trace = os.environ.get("KERNEL_TRACE", "0") == "1"
    res = bass_utils.run_bass_kernel_spmd(nc, in_maps,
                                          core_ids=list(range(NCORES)),
                                          trace=trace)
    if trace:
        print("exec_time_ns:", res.exec_time_ns,
              "mean:", res.mean_exec_time_ns,
              "core:", res.max_exec_time_core_id)
    outs = []
    for r in res.results:
        a = r["out"].reshape(NBP, 128, 2, FC, H)
        outs.append(a.transpose(0, 2, 4, 3, 1).reshape(BPC, H, F))
    return np.ascontiguousarray(np.concatenate(outs, axis=0), dtype=np.float32)


# revision 22
# speedup vs baseline: 2.6203x; 1.1568x over previous
"""Trainium2 Bass kernel for nn_AttentionModule (sparse_attention).

Reference computation:
  q = tanh(einsum('hde,be->hbd', Query, x))          H=8 D=256 E=1536
  k = tanh(einsum('hdf,blf->hbld', Key, bank))       B=64 L=256 F=768
  s = einsum('hbld,hbd->hbl', k, q)  masked softmax over l
  out = LeakyReLU_0.4(einsum('hbl,blf->bhf', attn, bank))

Strategy (data-parallel over batch B, 8 b's per core):
 * Mask compaction: the 0/1 mask keeps <=147 of 256 bank columns per b, so
   the host gathers unmasked columns and pads to LP=148.  Padding columns
   get a -1e4 additive score bias (exp -> 0) injected as an extra matmul.
 * The dominant k-matmul runs as error-compensated fp8 (e4m3): with
   Key*32 ~ K8 + Kr and bank*8 ~ B8 + Br, kraw = K8B8 + K8Br + KrB8
   (the fp8*fp8 residual cross term is negligible).  All three terms share
   one power-of-two scale, folded into the tanh eviction's `scale`.  Each
   product pair runs as a DoubleRow matmul (2 K-tiles per instruction).
 * Narrow dims (batch 8, heads 8) ride in the moving dimension: q, score,
   and emb matmuls cost ap_size 8 or 1 per instruction instead of 256-512.
 * Softmax skips max-subtraction (|score| < 40, safe in f32) so scores can
   stay in [l, h] layout; 1/z is broadcast to [f, h] via a ones-matmul and
   applied together with LeakyReLU on the vector engine.
 * All DMA streams are host-pre-swizzled to the exact SBUF layout
   ([128, X] row-major, contiguous >=512B lines); outputs are gathered as
   [f, (b2, fc, h)] tiles and transposed on the host.
"""

import os
import numpy as np
import ml_dtypes

import concourse.bass as bass  # noqa: F401
import concourse.mybir as mybir
import concourse.tile as tile
from concourse import bacc, bass_utils

F32 = mybir.dt.float32
F16 = mybir.dt.float16
BF16 = mybir.dt.bfloat16
FP8 = mybir.dt.float8e4
AF = mybir.ActivationFunctionType
ALU = mybir.AluOpType
DR = mybir.MatmulPerfMode.DoubleRow

H, D, E, F = 8, 256, 1536, 768
B, L = 64, 256
NCORES = 8
BPC = B // NCORES          # 8 b's per core
NBP = BPC // 2             # 4 b-pairs per core
EC, FC, DC = E // 128, F // 128, D // 128   # 12, 6, 2
# Per-bp padded unmasked-column counts: the host sorts the 64 b's by count
# and fills bp0 slots with the 16 largest, so later bps get shorter l'.
# Defaults match the fixed harness input (axon-jax PRNG).
LPS_DEFAULT = (152, 132, 128, 124)
SK, SB = 32.0, 8.0         # fp8 pre-scales for Key / bank (powers of two)

# f16 fallback for the k-matmul (accuracy reference / debugging)
K16 = os.environ.get("KERNEL_K16", "0") == "1"


def _build_program(lps=LPS_DEFAULT):
    assert all(lp % 2 == 0 for lp in lps)
    lhs_ = [lp // 2 for lp in lps]     # l-chunks: two per b
    lpps = [2 * lp for lp in lps]      # (b2, l') columns per (h, dc) group
    nsk = 1 if K16 else 2              # fp8: [K8, Kr] / [Br, B8] stream pairs
    ktdt = F16 if K16 else FP8
    kt_cols = nsk * FC * D             # per-h Key cols
    bkt_cols = [nsk * FC * w for w in lpps]   # per-bp bankT cols
    bkt_off = np.cumsum([0] + bkt_cols).tolist()
    bkn_off = np.cumsum([0] + [2 * lh for lh in lhs_]).tolist()
    sb_off = np.cumsum([0] + [4 * lh for lh in lhs_]).tolist()
    tanh_scale = 1.0 if K16 else 1.0 / (SK * SB)

    nc = bacc.Bacc("TRN2", target_bir_lowering=False, debug=False,
                   enable_asserts=False, num_devices=NCORES)
    qt = nc.dram_tensor("qt", [H, 128, EC * D], F16, kind="ExternalInput").ap()
    xt = nc.dram_tensor("xt", [128, EC * BPC], F16, kind="ExternalInput").ap()
    kt = nc.dram_tensor("kt", [H, 128, kt_cols], ktdt, kind="ExternalInput").ap()
    bkt = nc.dram_tensor("bkt", [128, bkt_off[-1]], ktdt, kind="ExternalInput").ap()
    bkn = nc.dram_tensor("bkn", [bkn_off[-1], 2 * F], BF16, kind="ExternalInput").ap()
    sbias = nc.dram_tensor("sbias", [1, sb_off[-1]], F32, kind="ExternalInput").ap()
    out = nc.dram_tensor("out", [NBP, 128, 2 * FC * H], F32, kind="ExternalOutput").ap()

    with tile.TileContext(nc) as tc:
        with tc.tile_pool(name="const", bufs=1) as cpool, \
             tc.tile_pool(name="weights", bufs=1) as wpool, \
             tc.tile_pool(name="bktp", bufs=1) as bpool, \
             tc.tile_pool(name="bknp", bufs=1) as npool, \
             tc.tile_pool(name="ksb", bufs=1) as kpool, \
             tc.tile_pool(name="small", bufs=2) as spool, \
             tc.tile_pool(name="psK", bufs=2, space="PSUM") as psK, \
             tc.tile_pool(name="psQ", bufs=1, space="PSUM") as psQ, \
             tc.tile_pool(name="psS", bufs=2, space="PSUM") as psS:

            # ---------------- DMA: priority order -------------------------
            xt_sb = cpool.tile([128, EC * BPC], F16)
            kt_sb = [wpool.tile([128, kt_cols], ktdt, name=f"kt{h}", tag=f"kt{h}")
                     for h in range(H)]
            qt_sb = [wpool.tile([128, EC * D], F16, name=f"qt{h}", tag=f"qt{h}")
                     for h in range(H)]
            bkt_t = [bpool.tile([128, bkt_cols[bp]], ktdt,
                                name=f"bkt{bp}", tag=f"bkt{bp}")
                     for bp in range(NBP)]
            bkn_t = [[npool.tile([lhs_[bp], 2 * F], BF16,
                                 name=f"bkn{bp}_{b2}", tag=f"bkn{bp}_{b2}")
                      for b2 in range(2)] for bp in range(NBP)]
            sb_sb = cpool.tile([1, sb_off[-1]], F32)
            onesb = cpool.tile([1, BPC], F32)
            ones_col = cpool.tile([lhs_[0], 1], BF16)
            ones128 = cpool.tile([1, 128], F32)

            def dma_bkt(bp, s=None):
                o = bkt_off[bp]
                w = bkt_cols[bp]
                if s is None:
                    nc.sync.dma_start(bkt_t[bp][:], bkt[:, o:o + w])
                else:
                    h2 = w // nsk
                    nc.sync.dma_start(bkt_t[bp][:, s * h2:(s + 1) * h2],
                                      bkt[:, o + s * h2:o + (s + 1) * h2])

            # kt0 K8-half and bkt0 B8-half first so T1 matmuls start early
            hk = kt_cols // nsk
            nc.sync.dma_start(kt_sb[0][:, 0:hk], kt[0, :, 0:hk])
            if nsk == 2:
                dma_bkt(0, 1)
                dma_bkt(0, 0)
                nc.sync.dma_start(kt_sb[0][:, hk:2 * hk], kt[0, :, hk:2 * hk])
            else:
                dma_bkt(0)
            nc.sync.dma_start(xt_sb[:], xt)
            for h in range(1, H):
                nc.sync.dma_start(kt_sb[h][:], kt[h])
            nc.vector.memset(onesb[:], 1.0)
            nc.vector.memset(ones_col[:], 1.0)
            nc.vector.memset(ones128[:], 1.0)
            dma_bkt(1)
            for h in range(0, 4):
                nc.sync.dma_start(qt_sb[h][:], qt[h])
            dma_bkt(2)
            for h in range(4, H):
                nc.sync.dma_start(qt_sb[h][:], qt[h])
            dma_bkt(3)
            nc.sync.dma_start(sb_sb[:], sbias)
            for bp in range(NBP):
                for b2 in range(2):
                    r = bkn_off[bp] + b2 * lhs_[bp]
                    nc.sync.dma_start(bkn_t[bp][b2][:],
                                      bkn[r:r + lhs_[bp]])

            # ---------------- k = tanh(Key @ bankT), all bps --------------
            k_sb = {}

            def k_phase(bp):
                lpp = lpps[bp]
                vb = bkt_t[bp][:].rearrange("p (s ft c) -> p s ft c", s=nsk, ft=FC)
                for h in range(H):
                    vk = kt_sb[h][:].rearrange("p (s ft d) -> p s ft d", s=nsk, ft=FC)
                    ps = psK.tile([128, 1024], F32, name="psk", tag="psk")
                    for dc in range(DC):
                        g = ps[:, dc * 512:dc * 512 + lpp]
                        if K16:
                            for ft in range(FC):
                                nc.tensor.matmul(
                                    g, vk[:, 0, ft, dc * 128:(dc + 1) * 128],
                                    vb[:, 0, ft], start=(ft == 0),
                                    stop=(ft == FC - 1))
                        else:
                            # T1: K8.B8 over f-tile pairs
                            for p in range(FC // 2):
                                nc.tensor.matmul(
                                    g,
                                    vk[:, 0, 2 * p:2 * p + 2, dc * 128:(dc + 1) * 128],
                                    vb[:, 1, 2 * p:2 * p + 2],
                                    start=(p == 0), stop=False, perf_mode=DR)
                            # cross terms: K8.Br + Kr.B8 per f-tile
                            for ft in range(FC):
                                nc.tensor.matmul(
                                    g,
                                    vk[:, :, ft, dc * 128:(dc + 1) * 128],
                                    vb[:, :, ft],
                                    start=False, stop=(ft == FC - 1), perf_mode=DR)
                    kt_out = kpool.tile([128, 2 * lpp], F16,
                                        name=f"k{bp}_{h}", tag=f"k{bp}_{h}")
                    nc.scalar.activation(
                        kt_out[:].rearrange("p (a b) -> p a b", a=2),
                        ps[:].rearrange("p (a b) -> p a b", a=2)[:, :, 0:lpp],
                        AF.Tanh, scale=tanh_scale)
                    k_sb[(bp, h)] = kt_out

            for bp in range(NBP - 1):
                k_phase(bp)

            # ---------------- q = tanh(Query @ x), transposed -------------
            # (issued after k(2): qt has streamed in behind the k inputs, and
            # bp0-2 score pipelines then overlap the last k-phase)
            psq = psQ.tile([128, 128], F32)
            for h in range(H):
                vq = qt_sb[h][:].rearrange("p (ec d) -> p ec d", ec=EC)
                for dc in range(DC):
                    g = psq[:, (h * DC + dc) * BPC:(h * DC + dc + 1) * BPC]
                    for ec in range(EC):
                        nc.tensor.matmul(
                            g, vq[:, ec, dc * 128:(dc + 1) * 128],
                            xt_sb[:, ec * BPC:(ec + 1) * BPC],
                            start=(ec == 0), stop=(ec == EC - 1))
            q_sb = cpool.tile([128, 128], F16)
            nc.scalar.activation(q_sb[:], psq[:], AF.Tanh)

            # ---------------- score / softmax / emb per bp ----------------
            def score_part(bp):
                lh, lp, lpp = lhs_[bp], lps[bp], lpps[bp]
                ps = psS.tile([128, 512], F32, name="mix", tag="mix")
                # scores: out [l', (b2, lc, h)], accumulate dc + pad bias
                for b2 in range(2):
                    for lc in range(2):
                        col = (b2 * 2 + lc) * H
                        boff = sb_off[bp] + (b2 * 2 + lc) * lh
                        nc.tensor.matmul(ps[0:lh, col:col + H],
                                         sb_sb[:, boff:boff + lh],
                                         onesb[:], start=True, stop=False)
                        for h in range(H):
                            for dc in range(DC):
                                nc.tensor.matmul(
                                    ps[0:lh, col + h:col + h + 1],
                                    k_sb[(bp, h)][:, dc * lpp + b2 * lp +
                                                  lc * lh:dc * lpp + b2 * lp +
                                                  lc * lh + lh],
                                    q_sb[:, (h * DC + dc) * BPC + bp * 2 + b2:
                                         (h * DC + dc) * BPC + bp * 2 + b2 + 1],
                                    start=False,
                                    stop=(h == H - 1 and dc == DC - 1))
                exp_t = spool.tile([lhs_[0], 4 * H], BF16, name="exp", tag="exp")
                nc.scalar.activation(exp_t[0:lh, :], ps[0:lh, 0:4 * H], AF.Exp)
                return ps, exp_t

            def rest_part(bp, ps, exp_t):
                lh = lhs_[bp]
                # z[b2, h] (cols 32:48): accumulate both lc chunks via
                # strided rhs slices so no cross-psum adds are needed
                ev = exp_t[0:lh, :].rearrange("p (b2 lc h) -> p b2 lc h",
                                              b2=2, lc=2)
                for lc in range(2):
                    nc.tensor.matmul(ps[0:1, 32:48], ones_col[0:lh, :],
                                     ev[:, :, lc],
                                     start=(lc == 0), stop=(lc == 1))
                rz = spool.tile([1, 2 * H], F32, name="rz", tag="rz")
                nc.vector.reciprocal(rz[:], ps[0:1, 32:48])
                # emb[f, (b2, fc, h)] (cols 192:288)
                for b2 in range(2):
                    for fc in range(FC):
                        col = 192 + (b2 * FC + fc) * H
                        for lc in range(2):
                            nc.tensor.matmul(
                                ps[:, col:col + H],
                                bkn_t[bp][b2][:, lc * F + fc * 128:
                                              lc * F + fc * 128 + 128],
                                exp_t[0:lh, (b2 * 2 + lc) * H:
                                      (b2 * 2 + lc + 1) * H],
                                start=(lc == 0), stop=(lc == 1))
                # rzb[f, (b2, h)] broadcast (cols 96:112)
                nc.tensor.matmul(ps[:, 96:112], ones128[:], rz[:],
                                 start=True, stop=True)
                # normalize (fc-broadcast view of rzb) + LeakyReLU on DVE
                rzb_sb = spool.tile([128, 2 * H], F32, name="rzb", tag="rzb")
                o1 = spool.tile([128, 2 * FC * H], F32, name="o1", tag="o1")
                o2 = spool.tile([128, 2 * FC * H], F32, name="o2", tag="o2")
                nc.vector.tensor_copy(rzb_sb[:], ps[:, 96:112])
                vb = rzb_sb[:].rearrange("p (b2 one h) -> p b2 one h",
                                         b2=2, one=1).broadcast_to([128, 2, FC, H])
                nc.vector.tensor_mul(
                    o1[:].rearrange("p (b2 fc h) -> p b2 fc h", b2=2, fc=FC),
                    ps[:, 192:288].rearrange("p (b2 fc h) -> p b2 fc h",
                                             b2=2, fc=FC), vb)
                nc.vector.scalar_tensor_tensor(o2[:], o1[:], 0.4, o1[:],
                                               ALU.mult, ALU.max)
                nc.sync.dma_start(out[bp], o2[:])

            pending = None
            for bp in range(NBP - 1):
                cur = (bp, *score_part(bp))
                if pending is not None:
                    rest_part(*pending)
                pending = cur
            k_phase(NBP - 1)
            rest_part(*pending)
            rest_part(NBP - 1, *score_part(NBP - 1))

    nc.finalize()
    return nc


def _slot_plan(mask):
    """Sort b's by unmasked count (desc); bp_j takes ranks [16j, 16j+16).
    Returns (perm, lps): perm[slot] = original b, slot = c*BPC + bp*2 + b2."""
    counts = mask.sum(axis=1)
    order = np.argsort(-counts, kind="stable")
    perm = np.empty(B, dtype=np.int64)
    for j in range(NBP):
        grp = order[16 * j:16 * (j + 1)]
        for c in range(NCORES):
            perm[c * BPC + j * 2] = grp[2 * c]
            perm[c * BPC + j * 2 + 1] = grp[2 * c + 1]
    lps = tuple(max(int(2 * ((counts[order[16 * j]] + 1) // 2)), 8)
                for j in range(NBP))
    return perm, lps


def _host_prep(x, bank, mask, Query, Key, perm, lps):
    x = np.asarray(x, dtype=np.float32)
    bank = np.asarray(bank, dtype=np.float32)
    mask = np.asarray(mask)
    Query = np.asarray(Query, dtype=np.float32)
    Key = np.asarray(Key, dtype=np.float32)
    e4 = ml_dtypes.float8_e4m3
    lhs_ = [lp // 2 for lp in lps]

    # q path: f16, host-transposed (slot-ordered x)
    xs = x[perm]
    qt = np.ascontiguousarray(Query.transpose(0, 2, 1)).reshape(
        H, EC, 128, D).transpose(0, 2, 1, 3).reshape(H, 128, EC * D)
    qt = qt.astype(np.float16)

    def swz_key(Kt):  # [H, D, F] -> [H, 128(f), FC, D]
        t = np.ascontiguousarray(Kt.transpose(0, 2, 1))
        return t.reshape(H, FC, 128, D).transpose(0, 2, 1, 3)

    if K16:
        kt = swz_key(Key).reshape(H, 128, FC * D).astype(np.float16)
    else:
        Ks = Key * SK
        K8 = Ks.astype(e4)
        Kr = (Ks - K8.astype(np.float32)).astype(e4)
        kt = np.stack([swz_key(K8.astype(np.float32)),
                       swz_key(Kr.astype(np.float32))], axis=2)
        kt = kt.reshape(H, 128, 2 * FC * D).astype(e4)

    # per-(core, bp) compacted bank streams, concatenated along columns/rows
    nsk = 1 if K16 else 2
    bdt = np.float16 if K16 else e4
    bkt_cols = sum(nsk * FC * 2 * lp for lp in lps)
    in_maps = []
    for c in range(NCORES):
        bkt_c = np.zeros((128, bkt_cols), dtype=bdt)
        bkn_rows = []
        sb_c = []
        col = 0
        for bp in range(NBP):
            lp, lh = lps[bp], lhs_[bp]
            bc = np.zeros((2, lp, F), dtype=np.float32)
            bias = np.zeros((2, lp), dtype=np.float32)
            for b2 in range(2):
                bsrc = perm[c * BPC + bp * 2 + b2]
                idx = np.nonzero(mask[bsrc])[0]
                bc[b2, :len(idx)] = bank[bsrc, idx]
                bias[b2, len(idx):] = -10000.0
            # bankT swizzle: [2, lp, F] -> [128(f), s?, FC, 2, lp]
            t = np.ascontiguousarray(bc.transpose(0, 2, 1))     # [2, F, lp]
            t = t.reshape(2, FC, 128, lp).transpose(2, 1, 0, 3)  # [128,FC,2,lp]
            if K16:
                blk = t.reshape(128, FC * 2 * lp).astype(bdt)
            else:
                ts = t * SB
                t8 = ts.astype(e4)
                tr = (ts - t8.astype(np.float32)).astype(e4)
                blk = np.stack([tr, t8.astype(e4)], axis=1).reshape(
                    128, 2 * FC * 2 * lp)
            w = nsk * FC * 2 * lp
            bkt_c[:, col:col + w] = blk
            col += w
            bkn_rows.append(bc.reshape(2, 2, lh, F).transpose(0, 2, 1, 3)
                            .reshape(2 * lh, 2 * F))
            sb_c.append(bias.reshape(4 * lh))
        in_maps.append({
            "qt": qt,
            "xt": np.ascontiguousarray(
                xs[c * BPC:(c + 1) * BPC].T.reshape(EC, 128, BPC)
                .transpose(1, 0, 2).reshape(128, EC * BPC)).astype(np.float16),
            "kt": kt,
            "bkt": bkt_c,
            "bkn": np.ascontiguousarray(np.concatenate(bkn_rows, axis=0))
            .astype(ml_dtypes.bfloat16),
            "sbias": np.concatenate(sb_c)[None, :].astype(np.float32),
        })
    return in_maps


_NC_CACHE = {}


def kernel(x, bank, mask, Query, Key):
    mask = np.asarray(mask)
    perm, lps = _slot_plan(mask)
    if lps not in _NC_CACHE:
        _NC_CACHE[lps] = _build_program(lps)
    nc = _NC_CACHE[lps]
    in_maps = _host_prep(x, bank, mask, Query, Key, perm, lps)

    trace = os.environ.get("KERNEL_TRACE", "0") == "1"
    res = bass_utils.run_bass_kernel_spmd(nc, in_maps,
                                          core_ids=list(range(NCORES)),
                                          trace=trace)
    if trace:
        print("exec_time_ns:", res.exec_time_ns,
              "mean:", res.mean_exec_time_ns,
              "core:", res.max_exec_time_core_id)
    full = np.empty((B, H, F), dtype=np.float32)
    for c, r in enumerate(res.results):
        a = r["out"].reshape(NBP, 128, 2, FC, H)
        full[perm[c * BPC:(c + 1) * BPC]] = (
            a.transpose(0, 2, 4, 3, 1).reshape(BPC, H, F))
    return np.ascontiguousarray(full)


# revision 26
# speedup vs baseline: 2.6697x; 1.0189x over previous
"""Trainium2 Bass kernel for nn_AttentionModule (sparse_attention).

Reference computation:
  q = tanh(einsum('hde,be->hbd', Query, x))          H=8 D=256 E=1536
  k = tanh(einsum('hdf,blf->hbld', Key, bank))       B=64 L=256 F=768
  s = einsum('hbld,hbd->hbl', k, q)  masked softmax over l
  out = LeakyReLU_0.4(einsum('hbl,blf->bhf', attn, bank))

Strategy (data-parallel over batch B, 8 b's per core):
 * Mask compaction: the 0/1 mask keeps <=147 of 256 bank columns per b, so
   the host gathers unmasked columns and pads to LP=148.  Padding columns
   get a -1e4 additive score bias (exp -> 0) injected as an extra matmul.
 * The dominant k-matmul runs as error-compensated fp8 (e4m3): with
   Key*32 ~ K8 + Kr and bank*8 ~ B8 + Br, kraw = K8B8 + K8Br + KrB8
   (the fp8*fp8 residual cross term is negligible).  All three terms share
   one power-of-two scale, folded into the tanh eviction's `scale`.  Each
   product pair runs as a DoubleRow matmul (2 K-tiles per instruction).
 * Narrow dims (batch 8, heads 8) ride in the moving dimension: q, score,
   and emb matmuls cost ap_size 8 or 1 per instruction instead of 256-512.
 * Softmax skips max-subtraction (|score| < 40, safe in f32) so scores can
   stay in [l, h] layout; 1/z is broadcast to [f, h] via a ones-matmul and
   applied together with LeakyReLU on the vector engine.
 * All DMA streams are host-pre-swizzled to the exact SBUF layout
   ([128, X] row-major, contiguous >=512B lines); outputs are gathered as
   [f, (b2, fc, h)] tiles and transposed on the host.
"""

import os
import numpy as np
import ml_dtypes

import concourse.bass as bass  # noqa: F401
import concourse.mybir as mybir
import concourse.tile as tile
from concourse import bacc, bass_utils

F32 = mybir.dt.float32
F16 = mybir.dt.float16
BF16 = mybir.dt.bfloat16
FP8 = mybir.dt.float8e4
AF = mybir.ActivationFunctionType
ALU = mybir.AluOpType
DR = mybir.MatmulPerfMode.DoubleRow

H, D, E, F = 8, 256, 1536, 768
B, L = 64, 256
NCORES = 8
BPC = B // NCORES          # 8 b's per core
NBP = BPC // 2             # 4 b-pairs per core
EC, FC, DC = E // 128, F // 128, D // 128   # 12, 6, 2
# Per-bp padded unmasked-column counts: the host sorts the 64 b's by count
# and fills bp0 slots with the 16 largest, so later bps get shorter l'.
# Defaults match the fixed harness input (axon-jax PRNG).
LPS_DEFAULT = (152, 132, 128, 124)
SK, SB = 32.0, 8.0         # fp8 pre-scales for Key / bank (powers of two)

# f16 fallback for the k-matmul (accuracy reference / debugging)
K16 = os.environ.get("KERNEL_K16", "0") == "1"


def _build_program(lps=LPS_DEFAULT):
    assert all(lp % 2 == 0 for lp in lps)
    lhs_ = [lp // 2 for lp in lps]     # l-chunks: two per b
    lpps = [2 * lp for lp in lps]      # (b2, l') columns per (h, dc) group
    nsk = 1 if K16 else 2              # fp8: [K8, Kr] / [Br, B8] stream pairs
    ktdt = F16 if K16 else FP8
    kt_cols = nsk * FC * D             # per-h Key cols
    bkt_cols = [nsk * FC * w for w in lpps]   # per-bp bankT cols
    bkt_off = np.cumsum([0] + bkt_cols).tolist()
    bkn_off = np.cumsum([0] + [2 * lh for lh in lhs_]).tolist()
    sb_off = np.cumsum([0] + [4 * lh for lh in lhs_]).tolist()
    tanh_scale = 1.0 if K16 else 1.0 / (SK * SB)

    nc = bacc.Bacc("TRN2", target_bir_lowering=False, debug=False,
                   enable_asserts=False, num_devices=NCORES)
    qt = nc.dram_tensor("qt", [H, 128, EC * D], F16, kind="ExternalInput").ap()
    xt = nc.dram_tensor("xt", [128, EC * BPC], F16, kind="ExternalInput").ap()
    kt = nc.dram_tensor("kt", [H, 128, kt_cols], ktdt, kind="ExternalInput").ap()
    bkt = nc.dram_tensor("bkt", [128, bkt_off[-1]], ktdt, kind="ExternalInput").ap()
    bkn = nc.dram_tensor("bkn", [bkn_off[-1], 2 * F], BF16, kind="ExternalInput").ap()
    sbias = nc.dram_tensor("sbias", [1, sb_off[-1]], F32, kind="ExternalInput").ap()
    out = nc.dram_tensor("out", [NBP, 128, 2 * FC * H], F32, kind="ExternalOutput").ap()

    with tile.TileContext(nc) as tc:
        with tc.tile_pool(name="const", bufs=1) as cpool, \
             tc.tile_pool(name="weights", bufs=1) as wpool, \
             tc.tile_pool(name="bktp", bufs=1) as bpool, \
             tc.tile_pool(name="bknp", bufs=1) as npool, \
             tc.tile_pool(name="ksb", bufs=1) as kpool, \
             tc.tile_pool(name="small", bufs=2) as spool, \
             tc.tile_pool(name="psK", bufs=2, space="PSUM") as psK, \
             tc.tile_pool(name="psQ", bufs=1, space="PSUM") as psQ, \
             tc.tile_pool(name="psS", bufs=2, space="PSUM") as psS:

            # ---------------- DMA: priority order -------------------------
            xt_sb = cpool.tile([128, EC * BPC], F16)
            kt_sb = [wpool.tile([128, kt_cols], ktdt, name=f"kt{h}", tag=f"kt{h}")
                     for h in range(H)]
            qt_sb = [wpool.tile([128, EC * D], F16, name=f"qt{h}", tag=f"qt{h}")
                     for h in range(H)]
            bkt_t = [bpool.tile([128, bkt_cols[bp]], ktdt,
                                name=f"bkt{bp}", tag=f"bkt{bp}")
                     for bp in range(NBP)]
            bkn_t = [[npool.tile([lhs_[bp], 2 * F], BF16,
                                 name=f"bkn{bp}_{b2}", tag=f"bkn{bp}_{b2}")
                      for b2 in range(2)] for bp in range(NBP)]
            sb_sb = cpool.tile([1, sb_off[-1]], F32)
            onesb = cpool.tile([1, BPC], F32)
            ones_col = cpool.tile([lhs_[0], 1], BF16)
            ones128 = cpool.tile([1, 128], F32)

            def dma_bkt(bp, s=None):
                o = bkt_off[bp]
                w = bkt_cols[bp]
                if s is None:
                    nc.sync.dma_start(bkt_t[bp][:], bkt[:, o:o + w])
                else:
                    h2 = w // nsk
                    nc.sync.dma_start(bkt_t[bp][:, s * h2:(s + 1) * h2],
                                      bkt[:, o + s * h2:o + (s + 1) * h2])

            # kt0 K8-half and bkt0 B8-half first so T1 matmuls start early
            hk = kt_cols // nsk
            nc.sync.dma_start(kt_sb[0][:, 0:hk], kt[0, :, 0:hk])
            if nsk == 2:
                dma_bkt(0, 1)
                nc.sync.dma_start(kt_sb[1][:], kt[1])
                dma_bkt(0, 0)
                nc.sync.dma_start(kt_sb[0][:, hk:2 * hk], kt[0, :, hk:2 * hk])
            else:
                dma_bkt(0)
                nc.sync.dma_start(kt_sb[1][:], kt[1])
            nc.sync.dma_start(xt_sb[:], xt)
            for h in range(2, H):
                nc.sync.dma_start(kt_sb[h][:], kt[h])
            nc.vector.memset(onesb[:], 1.0)
            nc.vector.memset(ones_col[:], 1.0)
            nc.vector.memset(ones128[:], 1.0)
            dma_bkt(1)
            for h in range(0, 2):
                nc.sync.dma_start(qt_sb[h][:], qt[h])
            dma_bkt(2)
            for h in range(2, 5):
                nc.sync.dma_start(qt_sb[h][:], qt[h])
            dma_bkt(3)
            for h in range(5, H):
                nc.sync.dma_start(qt_sb[h][:], qt[h])
            nc.sync.dma_start(sb_sb[:], sbias)
            for bp in range(NBP):
                for b2 in range(2):
                    r = bkn_off[bp] + b2 * lhs_[bp]
                    nc.sync.dma_start(bkn_t[bp][b2][:],
                                      bkn[r:r + lhs_[bp]])

            # ---------------- k = tanh(Key @ bankT), all bps --------------
            k_sb = {}

            def k_phase(bp, warm=False):
                lpp = lpps[bp]
                vb = bkt_t[bp][:].rearrange("p (s ft c) -> p s ft c", s=nsk, ft=FC)

                def t1_mms(h, ps):
                    vk = kt_sb[h][:].rearrange("p (s ft d) -> p s ft d",
                                               s=nsk, ft=FC)
                    for dc in range(DC):
                        g = ps[:, dc * 512:dc * 512 + lpp]
                        if K16:
                            for ft in range(FC):
                                nc.tensor.matmul(
                                    g, vk[:, 0, ft, dc * 128:(dc + 1) * 128],
                                    vb[:, 0, ft], start=(ft == 0),
                                    stop=(ft == FC - 1))
                        else:
                            for p in range(FC // 2):
                                nc.tensor.matmul(
                                    g,
                                    vk[:, 0, 2 * p:2 * p + 2,
                                       dc * 128:(dc + 1) * 128],
                                    vb[:, 1, 2 * p:2 * p + 2],
                                    start=(p == 0), stop=False, perf_mode=DR)

                def cross_evict(h, ps):
                    vk = kt_sb[h][:].rearrange("p (s ft d) -> p s ft d",
                                               s=nsk, ft=FC)
                    if not K16:
                        for dc in range(DC):
                            g = ps[:, dc * 512:dc * 512 + lpp]
                            # cross terms: K8.Br + Kr.B8 per f-tile
                            for ft in range(FC):
                                nc.tensor.matmul(
                                    g,
                                    vk[:, :, ft, dc * 128:(dc + 1) * 128],
                                    vb[:, :, ft],
                                    start=False, stop=(ft == FC - 1),
                                    perf_mode=DR)
                    kt_out = kpool.tile([128, 2 * lpp], F16,
                                        name=f"k{bp}_{h}", tag=f"k{bp}_{h}")
                    nc.scalar.activation(
                        kt_out[:].rearrange("p (a b) -> p a b", a=2),
                        ps[:].rearrange("p (a b) -> p a b", a=2)[:, :, 0:lpp],
                        AF.Tanh, scale=tanh_scale)
                    k_sb[(bp, h)] = kt_out

                tiles = {}
                start_h = 0
                if warm and not K16:
                    # first two heads: all T1 (needs only K8/B8 halves)
                    # before any cross work, so PE starts as data trickles in
                    for h in range(2):
                        tiles[h] = psK.tile([128, 1024], F32,
                                            name="psk", tag="psk")
                        t1_mms(h, tiles[h])
                    for h in range(2):
                        cross_evict(h, tiles.pop(h))
                    start_h = 2
                for h in range(start_h, H):
                    ps = psK.tile([128, 1024], F32, name="psk", tag="psk")
                    t1_mms(h, ps)
                    cross_evict(h, ps)

            k_phase(0, warm=True)
            for bp in range(1, NBP - 1):
                k_phase(bp)

            # ---------------- q = tanh(Query @ x), transposed -------------
            # (issued after k(2): qt has streamed in behind the k inputs, and
            # bp0-2 score pipelines then overlap the last k-phase)
            psq = psQ.tile([128, 128], F32)
            for h in range(H):
                vq = qt_sb[h][:].rearrange("p (ec d) -> p ec d", ec=EC)
                for dc in range(DC):
                    g = psq[:, (h * DC + dc) * BPC:(h * DC + dc + 1) * BPC]
                    for ec in range(EC):
                        nc.tensor.matmul(
                            g, vq[:, ec, dc * 128:(dc + 1) * 128],
                            xt_sb[:, ec * BPC:(ec + 1) * BPC],
                            start=(ec == 0), stop=(ec == EC - 1))
            q_sb = cpool.tile([128, 128], F16)
            nc.scalar.activation(q_sb[:], psq[:], AF.Tanh)

            # ---------------- score / softmax / emb per bp ----------------
            def score_part(bp):
                lh, lp, lpp = lhs_[bp], lps[bp], lpps[bp]
                ps = psS.tile([128, 512], F32, name="mix", tag="mix")
                # scores: out [l', (b2, lc, h)], accumulate dc + pad bias
                for b2 in range(2):
                    for lc in range(2):
                        col = (b2 * 2 + lc) * H
                        boff = sb_off[bp] + (b2 * 2 + lc) * lh
                        nc.tensor.matmul(ps[0:lh, col:col + H],
                                         sb_sb[:, boff:boff + lh],
                                         onesb[:], start=True, stop=False)
                        for h in range(H):
                            for dc in range(DC):
                                nc.tensor.matmul(
                                    ps[0:lh, col + h:col + h + 1],
                                    k_sb[(bp, h)][:, dc * lpp + b2 * lp +
                                                  lc * lh:dc * lpp + b2 * lp +
                                                  lc * lh + lh],
                                    q_sb[:, (h * DC + dc) * BPC + bp * 2 + b2:
                                         (h * DC + dc) * BPC + bp * 2 + b2 + 1],
                                    start=False,
                                    stop=(h == H - 1 and dc == DC - 1))
                exp_t = spool.tile([lhs_[0], 4 * H], BF16, name="exp", tag="exp")
                nc.scalar.activation(exp_t[0:lh, :], ps[0:lh, 0:4 * H], AF.Exp)
                return ps, exp_t

            def rest_part(bp, ps, exp_t):
                lh = lhs_[bp]
                # z[b2, h] (cols 32:48): accumulate both lc chunks via
                # strided rhs slices so no cross-psum adds are needed
                ev = exp_t[0:lh, :].rearrange("p (b2 lc h) -> p b2 lc h",
                                              b2=2, lc=2)
                for lc in range(2):
                    nc.tensor.matmul(ps[0:1, 32:48], ones_col[0:lh, :],
                                     ev[:, :, lc],
                                     start=(lc == 0), stop=(lc == 1))
                rz = spool.tile([1, 2 * H], F32, name="rz", tag="rz")
                nc.vector.reciprocal(rz[:], ps[0:1, 32:48])
                # emb[f, (b2, fc, h)] (cols 192:288)
                for b2 in range(2):
                    for fc in range(FC):
                        col = 192 + (b2 * FC + fc) * H
                        for lc in range(2):
                            nc.tensor.matmul(
                                ps[:, col:col + H],
                                bkn_t[bp][b2][:, lc * F + fc * 128:
                                              lc * F + fc * 128 + 128],
                                exp_t[0:lh, (b2 * 2 + lc) * H:
                                      (b2 * 2 + lc + 1) * H],
                                start=(lc == 0), stop=(lc == 1))
                # rzb[f, (b2, h)] broadcast (cols 96:112)
                nc.tensor.matmul(ps[:, 96:112], ones128[:], rz[:],
                                 start=True, stop=True)
                # normalize (fc-broadcast view of rzb) + LeakyReLU on DVE
                rzb_sb = spool.tile([128, 2 * H], F32, name="rzb", tag="rzb")
                o1 = spool.tile([128, 2 * FC * H], F32, name="o1", tag="o1")
                o2 = spool.tile([128, 2 * FC * H], F32, name="o2", tag="o2")
                nc.vector.tensor_copy(rzb_sb[:], ps[:, 96:112])
                vb = rzb_sb[:].rearrange("p (b2 one h) -> p b2 one h",
                                         b2=2, one=1).broadcast_to([128, 2, FC, H])
                nc.vector.tensor_mul(
                    o1[:].rearrange("p (b2 fc h) -> p b2 fc h", b2=2, fc=FC),
                    ps[:, 192:288].rearrange("p (b2 fc h) -> p b2 fc h",
                                             b2=2, fc=FC), vb)
                nc.vector.scalar_tensor_tensor(o2[:], o1[:], 0.4, o1[:],
                                               ALU.mult, ALU.max)
                nc.sync.dma_start(out[bp], o2[:])

            pending = None
            for bp in range(NBP - 1):
                cur = (bp, *score_part(bp))
                if pending is not None:
                    rest_part(*pending)
                pending = cur
            k_phase(NBP - 1)
            rest_part(*pending)
            rest_part(NBP - 1, *score_part(NBP - 1))

    nc.finalize()
    return nc


def _slot_plan(mask):
    """Sort b's by unmasked count (desc); bp_j takes ranks [16j, 16j+16).
    Returns (perm, lps): perm[slot] = original b, slot = c*BPC + bp*2 + b2."""
    counts = mask.sum(axis=1)
    order = np.argsort(-counts, kind="stable")
    perm = np.empty(B, dtype=np.int64)
    for j in range(NBP):
        grp = order[16 * j:16 * (j + 1)]
        for c in range(NCORES):
            perm[c * BPC + j * 2] = grp[2 * c]
            perm[c * BPC + j * 2 + 1] = grp[2 * c + 1]
    lps = tuple(max(int(2 * ((counts[order[16 * j]] + 1) // 2)), 8)
                for j in range(NBP))
    return perm, lps


def _host_prep(x, bank, mask, Query, Key, perm, lps):
    x = np.asarray(x, dtype=np.float32)
    bank = np.asarray(bank, dtype=np.float32)
    mask = np.asarray(mask)
    Query = np.asarray(Query, dtype=np.float32)
    Key = np.asarray(Key, dtype=np.float32)
    e4 = ml_dtypes.float8_e4m3
    lhs_ = [lp // 2 for lp in lps]

    # q path: f16, host-transposed (slot-ordered x)
    xs = x[perm]
    qt = np.ascontiguousarray(Query.transpose(0, 2, 1)).reshape(
        H, EC, 128, D).transpose(0, 2, 1, 3).reshape(H, 128, EC * D)
    qt = qt.astype(np.float16)

    def swz_key(Kt):  # [H, D, F] -> [H, 128(f), FC, D]
        t = np.ascontiguousarray(Kt.transpose(0, 2, 1))
        return t.reshape(H, FC, 128, D).transpose(0, 2, 1, 3)

    if K16:
        kt = swz_key(Key).reshape(H, 128, FC * D).astype(np.float16)
    else:
        Ks = Key * SK
        K8 = Ks.astype(e4)
        Kr = (Ks - K8.astype(np.float32)).astype(e4)
        kt = np.stack([swz_key(K8.astype(np.float32)),
                       swz_key(Kr.astype(np.float32))], axis=2)
        kt = kt.reshape(H, 128, 2 * FC * D).astype(e4)

    # per-(core, bp) compacted bank streams, concatenated along columns/rows
    nsk = 1 if K16 else 2
    bdt = np.float16 if K16 else e4
    bkt_cols = sum(nsk * FC * 2 * lp for lp in lps)
    in_maps = []
    for c in range(NCORES):
        bkt_c = np.zeros((128, bkt_cols), dtype=bdt)
        bkn_rows = []
        sb_c = []
        col = 0
        for bp in range(NBP):
            lp, lh = lps[bp], lhs_[bp]
            bc = np.zeros((2, lp, F), dtype=np.float32)
            bias = np.zeros((2, lp), dtype=np.float32)
            for b2 in range(2):
                bsrc = perm[c * BPC + bp * 2 + b2]
                idx = np.nonzero(mask[bsrc])[0]
                bc[b2, :len(idx)] = bank[bsrc, idx]
                bias[b2, len(idx):] = -10000.0
            # bankT swizzle: [2, lp, F] -> [128(f), s?, FC, 2, lp]
            t = np.ascontiguousarray(bc.transpose(0, 2, 1))     # [2, F, lp]
            t = t.reshape(2, FC, 128, lp).transpose(2, 1, 0, 3)  # [128,FC,2,lp]
            if K16:
                blk = t.reshape(128, FC * 2 * lp).astype(bdt)
            else:
                ts = t * SB
                t8 = ts.astype(e4)
                tr = (ts - t8.astype(np.float32)).astype(e4)
                blk = np.stack([tr, t8.astype(e4)], axis=1).reshape(
                    128, 2 * FC * 2 * lp)
            w = nsk * FC * 2 * lp
            bkt_c[:, col:col + w] = blk
            col += w
            bkn_rows.append(bc.reshape(2, 2, lh, F).transpose(0, 2, 1, 3)
                            .reshape(2 * lh, 2 * F))
            sb_c.append(bias.reshape(4 * lh))
        in_maps.append({
            "qt": qt,
            "xt": np.ascontiguousarray(
                xs[c * BPC:(c + 1) * BPC].T.reshape(EC, 128, BPC)
                .transpose(1, 0, 2).reshape(128, EC * BPC)).astype(np.float16),
            "kt": kt,
            "bkt": bkt_c,
            "bkn": np.ascontiguousarray(np.concatenate(bkn_rows, axis=0))
            .astype(ml_dtypes.bfloat16),
            "sbias": np.concatenate(sb_c)[None, :].astype(np.float32),
        })
    return in_maps


_NC_CACHE = {}


def kernel(x, bank, mask, Query, Key):
    mask = np.asarray(mask)
    perm, lps = _slot_plan(mask)
    if lps not in _NC_CACHE:
        _NC_CACHE[lps] = _build_program(lps)
    nc = _NC_CACHE[lps]
    in_maps = _host_prep(x, bank, mask, Query, Key, perm, lps)

    trace = os.environ.get("KERNEL_TRACE", "0") == "1"
    res = bass_utils.run_bass_kernel_spmd(nc, in_maps,
                                          core_ids=list(range(NCORES)),
                                          trace=trace)
    if trace:
        print("exec_time_ns:", res.exec_time_ns,
              "mean:", res.mean_exec_time_ns,
              "core:", res.max_exec_time_core_id)
    full = np.empty((B, H, F), dtype=np.float32)
    for c, r in enumerate(res.results):
        a = r["out"].reshape(NBP, 128, 2, FC, H)
        full[perm[c * BPC:(c + 1) * BPC]] = (
            a.transpose(0, 2, 4, 3, 1).reshape(BPC, H, F))
    return np.ascontiguousarray(full)
